# revision 18
# baseline (speedup 1.0000x reference)
"""MiniBatchDiscrimination Trainium2 kernel (8-core SPMD).

Reference computation:
    m = (x @ T).reshape(B, OUT_F, NUM_K)            # B=256, OUT_F=128, NUM_K=16
    dists = |m[None,:,:,:] - m[:,None,:,:]|         # [B, B, OUT_F, NUM_K]
    out = sum_i exp(-sum_k dists) - 1               # [B, OUT_F]
    return concat([x, out], axis=-1)                # [B, 640]

Strategy (per core, identical SPMD program, per-core data):
  * Each core owns JB=32 output rows (j); full m replicated (cheap GEMM).
  * m stored as [p=(f8,k), i, fo] with f = fo*8 + f8, p = f8*16 + k.
  * Max-trick: |a-b| = 2*max(a,b) - a - b, so with M_ij = sum_k max and
    s_i = sum_k m[i,f,k]:
        exp(-d_ij) = exp(-2*M_ij) * exp(s_i) * exp(s_j)
    This removes the abs pass entirely: DVE does ONE tensor_max per
    16-i block (2x mode), TensorE does the k-sum (same ones_k matmuls as
    the distance path), ACT does exp(-2*M), one small DVE multiply folds
    exp(s_i), and exp(s_j) is applied once at the end.
  * s_i is computed with the same ones_k matmul patterns on m itself;
    the diagonal stays near-exact because max(x,x)=x and both matmuls
    accumulate the identical 16 bf16 values in the same order.
  * sum over i: ones_acc matmul accumulating across all 16 blocks.
  * Host unshards: reshape to [32,128] per core, concat with x.
"""

import os
import numpy as np

import concourse.bass as bass
import concourse.tile as tile
from concourse import bacc, mybir

BF16 = mybir.dt.bfloat16
FP32 = mybir.dt.float32
NPBF16 = np.dtype(mybir.dt.np(BF16))

B = 256
IN_F = 512
OUT_F = 128
NUM_K = 16
N_CORES = 8
JB = B // N_CORES          # 32 j-rows owned per core
F8 = 8                     # f8 = f % 8   (partition group)
FO = OUT_F // F8           # 16 f_o values (free dim)
KC = IN_F // 128           # 4 contraction chunks for the GEMM
NBLK = B // 16             # 16 i-blocks of 16
# trailing i's per block whose max runs on GpSimd instead of DVE
GPSIMD_S = int(os.environ.get("GPSIMD_S", "0"))
# dummy matmuls issued during the input DMA wait to pull the PE out of its
# cold 1.2GHz HAM state before the real GEMM begins (~5us of filler)
WARM_MM = int(os.environ.get("WARM_MM", "12"))

EXP = mybir.ActivationFunctionType.Exp


def build_nc():
    nc = bacc.Bacc(name="minibatch_discrim")

    # host-prearranged [p, c, i]; columns B..B+JB repeat this core's own
    # j-columns so the diagonal of the pair matrix is exact.
    xT_d = nc.dram_tensor("xT", [128, KC, B + JB], BF16, kind="ExternalInput")
    # T_w[p, fo, c, n] = T[c*128+p, fo*128+n]; 16KB/partition, 2 DMAs.
    T_d = nc.dram_tensor("T_w", [128, FO * KC * 128], BF16, kind="ExternalInput")
    # [512 ones_k | 8 ones_acc | 8 ones_s8] packed into one DMA.
    ones_d = nc.dram_tensor("ones_pack", [128, 528], BF16, kind="ExternalInput")
    # fp32 identity for the s_i-fold matmul (fp32 keeps the diagonal exact)
    ident_d = nc.dram_tensor("ident", [128, 128], FP32, kind="ExternalInput")
    out_d = nc.dram_tensor("out_pair", [F8, JB, FO], FP32, kind="ExternalOutput")

    with tile.TileContext(nc) as tc:
        with (
            tc.tile_pool(name="const", bufs=1) as constp,
            tc.tile_pool(name="mm", bufs=1) as mmp,
            tc.tile_pool(name="gpsum", bufs=3, space=bass.MemorySpace.PSUM) as gps,
            tc.tile_pool(name="sp1", bufs=1, space=bass.MemorySpace.PSUM) as sp1,
            tc.tile_pool(name="sp2", bufs=1, space=bass.MemorySpace.PSUM) as sp2,
            tc.tile_pool(name="dpsum", bufs=2, space=bass.MemorySpace.PSUM) as dps,
            tc.tile_pool(name="apsum", bufs=1, space=bass.MemorySpace.PSUM) as aps,
            tc.tile_pool(name="work", bufs=2) as wp,
            tc.tile_pool(name="expp", bufs=3) as ep,
        ):
            # ---- inputs to SBUF: xT first (moving operand), then T halves.
            # Flat 2D slices keep the DMA element size large (8-16KB lines).
            xT_sb = constp.tile([128, KC, B + JB], BF16)
            nc.sync.dma_start(xT_sb[:], xT_d[:])
            T_sb = constp.tile([128, FO * KC * 128], BF16)
            qcol = (FO // 4) * KC * 128
            for ch in range(4):
                nc.sync.dma_start(
                    T_sb[:, ch * qcol:(ch + 1) * qcol],
                    T_d[:, ch * qcol:(ch + 1) * qcol],
                )
            ones_sb = constp.tile([128, 528], BF16)
            nc.sync.dma_start(ones_sb[:], ones_d[:])
            ident_sb = constp.tile([128, 128], FP32)
            nc.sync.dma_start(ident_sb[:], ident_d[:])

            zero_b = constp.tile([128, 1], FP32)
            nc.gpsimd.memset(zero_b[:], 0.0)
            # warm the ACT exp table while DMAs run
            warm = constp.tile([128, 1], FP32)
            nc.scalar.activation(warm[:], zero_b[:], EXP, bias=zero_b[:])

            # PE warm-up during the DMA wait: the HAM clock gate defaults to
            # 1.2GHz and needs ~3.4us of sustained matmul activity to release.
            if WARM_MM:
                wz = constp.tile([128, 512], BF16)
                nc.gpsimd.memset(wz[:], 0.0)
                wpd = gps.tile([128, B + JB], FP32, tag="gemm")
                for w in range(WARM_MM):
                    nc.tensor.matmul(
                        wpd[:], wz[:, :128], wz[:, :B + JB],
                        start=(w == 0), stop=(w == WARM_MM - 1),
                    )

            # ---- GEMM: m_full [p=(f8,k), i(288), fo] (cols B.. = own j) ----
            m_full = mmp.tile([128, B + JB, FO], BF16)
            for fo in range(FO):
                pm = gps.tile([128, B + JB], FP32, tag="gemm")
                for c in range(KC):
                    base = (fo * KC + c) * 128
                    nc.tensor.matmul(
                        pm[:],
                        T_sb[:, base:base + 128],
                        xT_sb[:, c, :],
                        start=(c == 0),
                        stop=(c == KC - 1),
                    )
                # strided-dst copies are slow (~1.5us) on either engine;
                # alternate ACT/DVE so the two streams overlap.
                if fo % 2 == 0:
                    nc.vector.tensor_copy(m_full[:, :, fo], pm[:])
                else:
                    nc.scalar.copy(m_full[:, :, fo], pm[:])

            # ---- s_i = sum_k m: same ones_k patterns as the dist matmuls ----
            # s_ps[p=(g,q,f8), blk, fo] = s_i for i = blk*16 + g*8 + q
            m_gq = m_full[:, :B, :].rearrange("p (blk gq) fo -> p gq blk fo", gq=16)
            s_ps = sp1.tile([128, NBLK, FO], FP32)
            for g in range(2):
                for q in range(8):
                    nc.tensor.matmul(
                        s_ps[g * 64:(g + 1) * 64],
                        ones_sb[:, q * 64:(q + 1) * 64],
                        m_gq[:, g * 8 + q],
                        start=(q == 0),
                        stop=(q == 7),
                    )
            # -(s_i)/2 in fp32; added into each dist PSUM tile by an identity
            # matmul so the exp(scale=-2) directly yields exp(-2M + s_i).
            s_half = mmp.tile([128, NBLK, FO], FP32)
            nc.vector.tensor_scalar_mul(s_half[:], s_ps[:], -0.5)

            # s_j for this core's own 32 columns -> c_sh [8, j, fo] fp32
            ssh_ps = sp2.tile([F8, JB, FO], FP32)
            nc.tensor.matmul(
                ssh_ps[:], ones_sb[:, 520:528], m_full[:, B:, :],
                start=True, stop=True,
            )
            c_sh = mmp.tile([F8, JB, FO], FP32)
            nc.scalar.activation(
                c_sh[:], ssh_ps[:], EXP, bias=zero_b[:F8], scale=1.0
            )

            # ---- main pairwise loop ----
            acc = aps.tile([F8, JB, FO], FP32)  # sum over i of exp-terms
            nd = 16 - GPSIMD_S
            for blk in range(NBLK):
                i0 = blk * 16
                mx = wp.tile([128, 16, JB, FO], BF16, tag="mx")
                nc.vector.tensor_max(
                    mx[:, :nd],
                    m_full[:, None, B:, :].broadcast_to([128, nd, JB, FO]),
                    m_full[:, i0:i0 + nd, None, :].broadcast_to(
                        [128, nd, JB, FO]
                    ),
                )
                if GPSIMD_S:
                    nc.gpsimd.tensor_max(
                        mx[:, nd:],
                        m_full[:, None, B:, :].broadcast_to(
                            [128, GPSIMD_S, JB, FO]
                        ),
                        m_full[:, i0 + nd:i0 + 16, None, :].broadcast_to(
                            [128, GPSIMD_S, JB, FO]
                        ),
                    )
                pd = dps.tile([128, JB, FO], FP32, tag="dist")
                for s in range(16):
                    g, q = s // 8, s % 8
                    nc.tensor.matmul(
                        pd[g * 64:(g + 1) * 64],
                        ones_sb[:, q * 64:(q + 1) * 64],
                        mx[:, s],
                        start=(q == 0),
                        stop=False,
                    )
                # pd += -s_i/2 (identity stationary; j-broadcast moving)
                nc.tensor.matmul(
                    pd[:],
                    ident_sb[:],
                    s_half[:, blk, None, :].broadcast_to([128, JB, FO]),
                    start=False,
                    stop=True,
                    skip_group_check=True,
                )
                et = ep.tile([128, JB, FO], BF16, tag="et")
                nc.scalar.activation(et[:], pd[:], EXP, bias=zero_b[:], scale=-2.0)
                nc.tensor.matmul(
                    acc[:],
                    ones_sb[:, 512:520],
                    et[:],
                    start=(blk == 0),
                    stop=(blk == NBLK - 1),
                    skip_group_check=True,
                )

            # ---- tail: * exp(s_j), subtract 1, store ----
            fin = mmp.tile([F8, JB, FO], FP32)
            nc.vector.tensor_mul(fin[:], acc[:], c_sh[:])
            fin2 = mmp.tile([F8, JB, FO], FP32)
            nc.vector.tensor_scalar_add(fin2[:], fin[:], -1.0)
            nc.sync.dma_start(out_d[:], fin2[:])

    nc.finalize()
    return nc


def make_in_maps(x: np.ndarray, T: np.ndarray):
    # xT_h[p, c, i] = x[i, c*128+p]
    xT_h = np.ascontiguousarray(
        x.T.astype(NPBF16).reshape(KC, 128, B).transpose(1, 0, 2)
    )
    T_b = np.ascontiguousarray(T).astype(NPBF16)           # [512, 2048]
    # T_w[p, fo, c, n] = T[c*128+p, fo*128+n]
    T_perm = np.ascontiguousarray(
        T_b.reshape(KC, 128, FO, 128).transpose(1, 2, 0, 3)
    ).reshape(128, FO * KC * 128)

    p = np.arange(128)[:, None]
    r = np.arange(F8)[None, :]
    ones_a = (p % 8 == r).astype(NPBF16)                   # [128, 8]
    ones_s8 = (p // 16 == r).astype(NPBF16)                # [128, 8]
    # ones_k[p, q8, q] = 1 iff q == q8*8 + p//16  (q in 0..63)
    q = np.arange(64)[None, None, :]
    s = np.arange(8)[None, :, None]
    ones_k = (q == s * 8 + p[:, :, None] // 16).astype(NPBF16).reshape(128, 512)
    ones_pack = np.ascontiguousarray(
        np.concatenate([ones_k, ones_a, ones_s8], axis=1)
    )

    in_maps = []
    for c in range(N_CORES):
        xTc = np.ascontiguousarray(np.concatenate(
            [xT_h, xT_h[:, :, c * JB:(c + 1) * JB]], axis=2
        ))
        in_maps.append({
            "xT": xTc,
            "T_w": T_perm,
            "ones_pack": ones_pack,
            "ident": np.eye(128, dtype=np.float32),
        })
    return in_maps


def assemble(x: np.ndarray, pair_parts) -> np.ndarray:
    """pair_parts: list of [8, JB, FO] fp32 per core -> full [B, IN_F+OUT_F]."""
    out = np.empty((B, IN_F + OUT_F), np.float32)
    out[:, :IN_F] = x
    for c, fp in enumerate(pair_parts):
        # fp[f8, j, fo] -> out[c*JB + j, IN_F + fo*8 + f8]
        blk = fp.reshape(F8, JB, FO).transpose(1, 2, 0).reshape(JB, OUT_F)
        out[c * JB:(c + 1) * JB, IN_F:] = blk
    return out


_NC_CACHE = None


def kernel(x: np.ndarray, T: np.ndarray) -> np.ndarray:
    global _NC_CACHE
    from concourse import bass_utils

    if _NC_CACHE is None:
        _NC_CACHE = build_nc()
    nc = _NC_CACHE
    in_maps = make_in_maps(np.asarray(x, np.float32), np.asarray(T, np.float32))
    res = bass_utils.run_bass_kernel_spmd(nc, in_maps, core_ids=list(range(N_CORES)))
    parts = [r["out_pair"].astype(np.float32) for r in res.results]
    return assemble(np.asarray(x, np.float32), parts)


# revision 20
# speedup vs baseline: 1.2012x; 1.2012x over previous
"""MiniBatchDiscrimination Trainium2 kernel (8-core SPMD).

Reference computation:
    m = (x @ T).reshape(B, OUT_F, NUM_K)            # B=256, OUT_F=128, NUM_K=16
    dists = |m[None,:,:,:] - m[:,None,:,:]|         # [B, B, OUT_F, NUM_K]
    out = sum_i exp(-sum_k dists) - 1               # [B, OUT_F]
    return concat([x, out], axis=-1)                # [B, 640]

Strategy (per core, identical SPMD program, per-core data):
  * Each core owns JB=32 output rows (j); full m replicated (cheap GEMM).
  * m stored as [p=(f8,k), i, fo] with f = fo*8 + f8, p = f8*16 + k.
  * Max-trick: |a-b| = 2*max(a,b) - a - b, so with M_ij = sum_k max and
    s_i = sum_k m[i,f,k]:
        exp(-d_ij) = exp(-2*M_ij) * exp(s_i) * exp(s_j)
    This removes the abs pass entirely: DVE does ONE tensor_max per
    16-i block (2x mode), TensorE does the k-sum (same ones_k matmuls as
    the distance path), ACT does exp(-2*M), one small DVE multiply folds
    exp(s_i), and exp(s_j) is applied once at the end.
  * s_i is computed with the same ones_k matmul patterns on m itself;
    the diagonal stays near-exact because max(x,x)=x and both matmuls
    accumulate the identical 16 bf16 values in the same order.
  * sum over i: ones_acc matmul accumulating across all 16 blocks.
  * Host unshards: reshape to [32,128] per core, concat with x.
"""

import os
import numpy as np

import concourse.bass as bass
import concourse.tile as tile
from concourse import bacc, mybir

BF16 = mybir.dt.bfloat16
FP32 = mybir.dt.float32
NPBF16 = np.dtype(mybir.dt.np(BF16))

B = 256
IN_F = 512
OUT_F = 128
NUM_K = 16
N_CORES = 8
JB = B // N_CORES          # 32 j-rows owned per core
F8 = 8                     # f8 = f % 8   (partition group)
FO = OUT_F // F8           # 16 f_o values (free dim)
KC = IN_F // 128           # 4 contraction chunks for the GEMM
NBLK = B // 16             # 16 i-blocks of 16
# trailing i's per block whose max runs on GpSimd instead of DVE
GPSIMD_S = int(os.environ.get("GPSIMD_S", "0"))
# dummy matmuls issued during the input DMA wait to pull the PE out of its
# cold 1.2GHz HAM state before the real GEMM begins (~5us of filler)
WARM_MM = int(os.environ.get("WARM_MM", "20"))

EXP = mybir.ActivationFunctionType.Exp


def build_nc():
    nc = bacc.Bacc(name="minibatch_discrim")

    # host-prearranged [p, c, i]; columns B..B+JB repeat this core's own
    # j-columns so the diagonal of the pair matrix is exact.
    xT_d = nc.dram_tensor("xT", [128, KC, B + JB], BF16, kind="ExternalInput")
    # T_w[p, fo, c, n] = T[c*128+p, fo*128+n]; 16KB/partition, 2 DMAs.
    T_d = nc.dram_tensor("T_w", [128, FO * KC * 128], BF16, kind="ExternalInput")
    # [512 ones_k | 8 ones_acc | 8 ones_s8] packed into one DMA.
    ones_d = nc.dram_tensor("ones_pack", [128, 528], BF16, kind="ExternalInput")
    # fp32 identity for the s_i-fold matmul (fp32 keeps the diagonal exact)
    ident_d = nc.dram_tensor("ident", [128, 128], FP32, kind="ExternalInput")
    out_d = nc.dram_tensor("out_pair", [F8, JB, FO], FP32, kind="ExternalOutput")

    with tile.TileContext(nc) as tc:
        with (
            tc.tile_pool(name="const", bufs=1) as constp,
            tc.tile_pool(name="mm", bufs=1) as mmp,
            tc.tile_pool(name="gpsum", bufs=3, space=bass.MemorySpace.PSUM) as gps,
            tc.tile_pool(name="sp1", bufs=1, space=bass.MemorySpace.PSUM) as sp1,
            tc.tile_pool(name="sp2", bufs=1, space=bass.MemorySpace.PSUM) as sp2,
            tc.tile_pool(name="dpsum", bufs=2, space=bass.MemorySpace.PSUM) as dps,
            tc.tile_pool(name="apsum", bufs=1, space=bass.MemorySpace.PSUM) as aps,
            tc.tile_pool(name="work", bufs=2) as wp,
            tc.tile_pool(name="expp", bufs=3) as ep,
        ):
            # ---- inputs to SBUF: xT first (moving operand), then T halves.
            # Flat 2D slices keep the DMA element size large (8-16KB lines).
            xT_sb = constp.tile([128, KC, B + JB], BF16)
            nc.sync.dma_start(xT_sb[:], xT_d[:])
            T_sb = constp.tile([128, FO * KC * 128], BF16)
            hcol = (FO // 2) * KC * 128
            nc.sync.dma_start(T_sb[:, :hcol], T_d[:, :hcol])
            nc.sync.dma_start(T_sb[:, hcol:], T_d[:, hcol:])
            ones_sb = constp.tile([128, 528], BF16)
            nc.sync.dma_start(ones_sb[:], ones_d[:])
            ident_sb = constp.tile([128, 128], FP32)
            nc.sync.dma_start(ident_sb[:], ident_d[:])

            zero_b = constp.tile([128, 1], FP32)
            nc.gpsimd.memset(zero_b[:], 0.0)
            # warm the ACT exp table while DMAs run
            warm = constp.tile([128, 1], FP32)
            nc.scalar.activation(warm[:], zero_b[:], EXP, bias=zero_b[:])

            # PE warm-up during the DMA wait: the HAM clock gate defaults to
            # 1.2GHz and needs ~3.4us of sustained matmul activity to release.
            if WARM_MM:
                wz = constp.tile([128, 512], BF16)
                nc.gpsimd.memset(wz[:], 0.0)
                wpd = gps.tile([128, B + JB], FP32, tag="gemm")
                for w in range(WARM_MM):
                    nc.tensor.matmul(
                        wpd[:], wz[:, :128], wz[:, :B + JB],
                        start=(w == 0), stop=(w == WARM_MM - 1),
                    )

            # ---- GEMM: m_full [p=(f8,k), i(288), fo] (cols B.. = own j) ----
            m_full = mmp.tile([128, B + JB, FO], BF16)
            for fo in range(FO):
                pm = gps.tile([128, B + JB], FP32, tag="gemm")
                for c in range(KC):
                    base = (fo * KC + c) * 128
                    nc.tensor.matmul(
                        pm[:],
                        T_sb[:, base:base + 128],
                        xT_sb[:, c, :],
                        start=(c == 0),
                        stop=(c == KC - 1),
                    )
                # strided-dst copies are slow (~1.5us) on either engine;
                # alternate ACT/DVE so the two streams overlap.
                if fo % 2 == 0:
                    nc.vector.tensor_copy(m_full[:, :, fo], pm[:])
                else:
                    nc.scalar.copy(m_full[:, :, fo], pm[:])

            # ---- s_i = sum_k m: same ones_k patterns as the dist matmuls ----
            # s_ps[p=(g,q,f8), blk, fo] = s_i for i = blk*16 + g*8 + q
            m_gq = m_full[:, :B, :].rearrange("p (blk gq) fo -> p gq blk fo", gq=16)
            s_ps = sp1.tile([128, NBLK, FO], FP32)
            for g in range(2):
                for q in range(8):
                    nc.tensor.matmul(
                        s_ps[g * 64:(g + 1) * 64],
                        ones_sb[:, q * 64:(q + 1) * 64],
                        m_gq[:, g * 8 + q],
                        start=(q == 0),
                        stop=(q == 7),
                    )
            # -(s_i)/2 in fp32; added into each dist PSUM tile by an identity
            # matmul so the exp(scale=-2) directly yields exp(-2M + s_i).
            s_half = mmp.tile([128, NBLK, FO], FP32)
            nc.vector.tensor_scalar_mul(s_half[:], s_ps[:], -0.5)

            # s_j for this core's own 32 columns -> c_sh [8, j, fo] fp32
            ssh_ps = sp2.tile([F8, JB, FO], FP32)
            nc.tensor.matmul(
                ssh_ps[:], ones_sb[:, 520:528], m_full[:, B:, :],
                start=True, stop=True,
            )
            c_sh = mmp.tile([F8, JB, FO], FP32)
            nc.scalar.activation(
                c_sh[:], ssh_ps[:], EXP, bias=zero_b[:F8], scale=1.0
            )

            # ---- main pairwise loop ----
            acc = aps.tile([F8, JB, FO], FP32)  # sum over i of exp-terms
            nd = 16 - GPSIMD_S
            for blk in range(NBLK):
                i0 = blk * 16
                mx = wp.tile([128, 16, JB, FO], BF16, tag="mx")
                nc.vector.tensor_max(
                    mx[:, :nd],
                    m_full[:, None, B:, :].broadcast_to([128, nd, JB, FO]),
                    m_full[:, i0:i0 + nd, None, :].broadcast_to(
                        [128, nd, JB, FO]
                    ),
                )
                if GPSIMD_S:
                    nc.gpsimd.tensor_max(
                        mx[:, nd:],
                        m_full[:, None, B:, :].broadcast_to(
                            [128, GPSIMD_S, JB, FO]
                        ),
                        m_full[:, i0 + nd:i0 + 16, None, :].broadcast_to(
                            [128, GPSIMD_S, JB, FO]
                        ),
                    )
                pd = dps.tile([128, JB, FO], FP32, tag="dist")
                for s in range(16):
                    g, q = s // 8, s % 8
                    nc.tensor.matmul(
                        pd[g * 64:(g + 1) * 64],
                        ones_sb[:, q * 64:(q + 1) * 64],
                        mx[:, s],
                        start=(q == 0),
                        stop=False,
                    )
                # pd += -s_i/2 (identity stationary; j-broadcast moving)
                nc.tensor.matmul(
                    pd[:],
                    ident_sb[:],
                    s_half[:, blk, None, :].broadcast_to([128, JB, FO]),
                    start=False,
                    stop=True,
                    skip_group_check=True,
                )
                et = ep.tile([128, JB, FO], BF16, tag="et")
                nc.scalar.activation(et[:], pd[:], EXP, bias=zero_b[:], scale=-2.0)
                nc.tensor.matmul(
                    acc[:],
                    ones_sb[:, 512:520],
                    et[:],
                    start=(blk == 0),
                    stop=(blk == NBLK - 1),
                    skip_group_check=True,
                )

            # ---- tail: * exp(s_j), subtract 1, store ----
            fin = mmp.tile([F8, JB, FO], FP32)
            nc.vector.tensor_mul(fin[:], acc[:], c_sh[:])
            fin2 = mmp.tile([F8, JB, FO], FP32)
            nc.vector.tensor_scalar_add(fin2[:], fin[:], -1.0)
            nc.sync.dma_start(out_d[:], fin2[:])

    nc.finalize()
    return nc


def make_in_maps(x: np.ndarray, T: np.ndarray):
    # xT_h[p, c, i] = x[i, c*128+p]
    xT_h = np.ascontiguousarray(
        x.T.astype(NPBF16).reshape(KC, 128, B).transpose(1, 0, 2)
    )
    T_b = np.ascontiguousarray(T).astype(NPBF16)           # [512, 2048]
    # T_w[p, fo, c, n] = T[c*128+p, fo*128+n]
    T_perm = np.ascontiguousarray(
        T_b.reshape(KC, 128, FO, 128).transpose(1, 2, 0, 3)
    ).reshape(128, FO * KC * 128)

    p = np.arange(128)[:, None]
    r = np.arange(F8)[None, :]
    ones_a = (p % 8 == r).astype(NPBF16)                   # [128, 8]
    ones_s8 = (p // 16 == r).astype(NPBF16)                # [128, 8]
    # ones_k[p, q8, q] = 1 iff q == q8*8 + p//16  (q in 0..63)
    q = np.arange(64)[None, None, :]
    s = np.arange(8)[None, :, None]
    ones_k = (q == s * 8 + p[:, :, None] // 16).astype(NPBF16).reshape(128, 512)
    ones_pack = np.ascontiguousarray(
        np.concatenate([ones_k, ones_a, ones_s8], axis=1)
    )

    in_maps = []
    for c in range(N_CORES):
        xTc = np.ascontiguousarray(np.concatenate(
            [xT_h, xT_h[:, :, c * JB:(c + 1) * JB]], axis=2
        ))
        in_maps.append({
            "xT": xTc,
            "T_w": T_perm,
            "ones_pack": ones_pack,
            "ident": np.eye(128, dtype=np.float32),
        })
    return in_maps


def assemble(x: np.ndarray, pair_parts) -> np.ndarray:
    """pair_parts: list of [8, JB, FO] fp32 per core -> full [B, IN_F+OUT_F]."""
    out = np.empty((B, IN_F + OUT_F), np.float32)
    out[:, :IN_F] = x
    for c, fp in enumerate(pair_parts):
        # fp[f8, j, fo] -> out[c*JB + j, IN_F + fo*8 + f8]
        blk = fp.reshape(F8, JB, FO).transpose(1, 2, 0).reshape(JB, OUT_F)
        out[c * JB:(c + 1) * JB, IN_F:] = blk
    return out


_NC_CACHE = None


def kernel(x: np.ndarray, T: np.ndarray) -> np.ndarray:
    global _NC_CACHE
    from concourse import bass_utils

    if _NC_CACHE is None:
        _NC_CACHE = build_nc()
    nc = _NC_CACHE
    in_maps = make_in_maps(np.asarray(x, np.float32), np.asarray(T, np.float32))
    res = bass_utils.run_bass_kernel_spmd(nc, in_maps, core_ids=list(range(N_CORES)))
    parts = [r["out_pair"].astype(np.float32) for r in res.results]
    return assemble(np.asarray(x, np.float32), parts)


# revision 21
# speedup vs baseline: 1.2022x; 1.0009x over previous
"""MiniBatchDiscrimination TRN2 kernel v2: triangle sharding (8-core SPMD).

Same max-trick math as kernel.py, but each (i-block, j-block) pair of the
symmetric BxB distance matrix is computed ONCE globally:

  * 16 row-blocks of 16. Core c owns j-blocks {c, c+8} (32 j-columns).
  * A tournament on the 8 residues orients every residue pair; core s
    computes i-block slots P_s = own 2 blocks + both blocks of every
    in-neighbor residue (8 or 10 real slots, padded to T=10).
  * Every block-pair {a,b} is covered exactly once: by the core owning b
    as a j-block if res(a) -> res(b), else by the core owning a.
  * Each computed tile (i-slot t x 32 j) yields BOTH output contributions:
      straight: sum_i exp(-d) -> rows of the j-block   (ones_acc matmul)
      mirror:   sum_j exp(-d) -> rows of the i-block   (DVE reduce over j)
    The host keeps only the owned (t, j-half) pieces and sums.
  * Both corrections are folded into the dist PSUM before exp:
      pd = M - s_i/2 - s_j/2   (identity matmul + f8-selector matmul)
    so et = exp(-2*pd) = exp(-d) directly; no tail corrections.
"""

import os
import numpy as np

import concourse.bass as bass
import concourse.tile as tile
from concourse import bacc, mybir

BF16 = mybir.dt.bfloat16
FP32 = mybir.dt.float32
NPBF16 = np.dtype(mybir.dt.np(BF16))

B = 256
IN_F = 512
OUT_F = 128
NUM_K = 16
N_CORES = 8
F8 = 8
FO = OUT_F // F8           # 16
KC = IN_F // 128           # 4
NB = 16                    # row blocks of 16
T = 10                     # i-block slots per core (padded)
MI = T * 16                # 160 i columns
MCOL = MI + 32             # + 32 own j columns (duplicated for exact diag)
WARM_MM = int(os.environ.get("WARM_MM", "20"))

EXP = mybir.ActivationFunctionType.Exp


# ---- tournament / coverage tables (host side) ----
def _edge_to(r, s):
    """True iff residue r -> residue s (core s computes i-blocks of r)."""
    d = (s - r) % 8
    return d in (1, 2, 3) or (d == 4 and r < 4)


def _slots_for_core(s):
    """Ordered i-block slot list for core s (len T, with pad dups)."""
    blocks = [s, s + 8]
    for r in range(8):
        if r != s and _edge_to(r, s):
            blocks += [r, r + 8]
    while len(blocks) < T:
        blocks.append(blocks[0])   # dummy pad (host ignores)
    assert len(blocks) == T, blocks
    return blocks


def _ownership():
    """Block pair {a,b} -> (core, slot, half, mirror?, iblk, jblk).

    The owning core computes tile (i-slot of iblk) x (j-half of jblk);
    straight goes to rows of jblk, mirror to rows of iblk.
    mirror? is False for a==b (mirror side would double count).
    """
    own = {}
    slot_tab = [_slots_for_core(s) for s in range(8)]

    def first_slot(c, a):
        return slot_tab[c].index(a)

    for a in range(NB):
        for b in range(a, NB):
            ra, rb = a % 8, b % 8
            if ra == rb:
                c = ra
                if a == b:
                    own[(a, b)] = (c, first_slot(c, a), a // 8, False, a, a)
                else:
                    lo, hi = (a, b) if a < b else (b, a)
                    own[(lo, hi)] = (
                        c, first_slot(c, lo), hi // 8, True, lo, hi
                    )
            elif _edge_to(ra, rb):
                c = rb
                own[(a, b)] = (c, first_slot(c, a), b // 8, True, a, b)
            else:
                c = ra
                own[(a, b)] = (c, first_slot(c, b), a // 8, True, b, a)
    return slot_tab, own


SLOT_TAB, OWN = _ownership()


def build_nc():
    nc = bacc.Bacc(name="minibatch_discrim2")

    xT_d = nc.dram_tensor("xT", [128, KC, MCOL], BF16, kind="ExternalInput")
    T_d = nc.dram_tensor("T_w", [128, FO * KC * 128], BF16, kind="ExternalInput")
    # [512 ones_k | 8 ones_acc | 8 pad | 128 sj_stat] = 656 cols
    ones_d = nc.dram_tensor("ones_pack", [128, 656], BF16, kind="ExternalInput")
    ident_d = nc.dram_tensor("ident", [128, 128], FP32, kind="ExternalInput")
    ostr_d = nc.dram_tensor("out_str", [F8, T, 32, FO], FP32, kind="ExternalOutput")
    omir_d = nc.dram_tensor("out_mir", [128, T, 2, FO], FP32, kind="ExternalOutput")

    with tile.TileContext(nc) as tc:
        with (
            tc.tile_pool(name="const", bufs=1) as constp,
            tc.tile_pool(name="mm", bufs=1) as mmp,
            tc.tile_pool(name="gpsum", bufs=3, space=bass.MemorySpace.PSUM) as gps,
            tc.tile_pool(name="sp1", bufs=1, space=bass.MemorySpace.PSUM) as sp1,
            tc.tile_pool(name="dpsum", bufs=2, space=bass.MemorySpace.PSUM) as dps,
            tc.tile_pool(name="apsum", bufs=2, space=bass.MemorySpace.PSUM) as aps,
            tc.tile_pool(name="work", bufs=2) as wp,
            tc.tile_pool(name="expp", bufs=3) as ep,
        ):
            xT_sb = constp.tile([128, KC, MCOL], BF16)
            nc.sync.dma_start(xT_sb[:], xT_d[:])
            T_sb = constp.tile([128, FO * KC * 128], BF16)
            hcol = (FO // 2) * KC * 128
            nc.sync.dma_start(T_sb[:, :hcol], T_d[:, :hcol])
            nc.sync.dma_start(T_sb[:, hcol:], T_d[:, hcol:])
            ones_sb = constp.tile([128, 656], BF16)
            nc.sync.dma_start(ones_sb[:], ones_d[:])
            ident_sb = constp.tile([128, 128], FP32)
            nc.sync.dma_start(ident_sb[:], ident_d[:])

            zero_b = constp.tile([128, 1], FP32)
            nc.gpsimd.memset(zero_b[:], 0.0)
            warm = constp.tile([128, 1], FP32)
            nc.scalar.activation(warm[:], zero_b[:], EXP, bias=zero_b[:])

            if WARM_MM:
                wz = constp.tile([128, 512], BF16)
                nc.gpsimd.memset(wz[:], 0.0)
                wpd = gps.tile([128, MCOL], FP32, tag="gemm")
                for w in range(WARM_MM):
                    nc.tensor.matmul(
                        wpd[:], wz[:, :128], wz[:, :MCOL],
                        start=(w == 0), stop=(w == WARM_MM - 1),
                    )

            # ---- GEMM: m_full [p=(f8,k), col(192), fo] ----
            m_full = mmp.tile([128, MCOL, FO], BF16)
            for fo in range(FO):
                pm = gps.tile([128, MCOL], FP32, tag="gemm")
                for c in range(KC):
                    base = (fo * KC + c) * 128
                    nc.tensor.matmul(
                        pm[:],
                        T_sb[:, base:base + 128],
                        xT_sb[:, c, :],
                        start=(c == 0),
                        stop=(c == KC - 1),
                    )
                if fo % 2 == 0:
                    nc.vector.tensor_copy(m_full[:, :, fo], pm[:])
                else:
                    nc.scalar.copy(m_full[:, :, fo], pm[:])

            # ---- s_i for the 10 i-slots ----
            m_gq = m_full[:, :MI, :].rearrange(
                "p (blk gq) fo -> p gq blk fo", gq=16
            )
            s_ps = sp1.tile([128, T, FO], FP32)
            for g in range(2):
                for q in range(8):
                    nc.tensor.matmul(
                        s_ps[g * 64:(g + 1) * 64],
                        ones_sb[:, q * 64:(q + 1) * 64],
                        m_gq[:, g * 8 + q],
                        start=(q == 0),
                        stop=(q == 7),
                    )
            s_half = mmp.tile([128, T, FO], FP32)
            nc.vector.tensor_scalar_mul(s_half[:], s_ps[:], -0.5)

            # staging for outputs
            str_stage = mmp.tile([F8, T, 32, FO], FP32)
            mir_stage = mmp.tile([128, T, 2, FO], FP32)

            # ---- main pairwise loop over the 10 i-slots ----
            for t in range(T):
                i0 = t * 16
                mx = wp.tile([128, 16, 32, FO], BF16, tag="mx")
                nc.vector.tensor_max(
                    mx[:],
                    m_full[:, None, MI:, :].broadcast_to([128, 16, 32, FO]),
                    m_full[:, i0:i0 + 16, None, :].broadcast_to(
                        [128, 16, 32, FO]
                    ),
                )
                pd = dps.tile([128, 32, FO], FP32, tag="dist")
                for s in range(16):
                    g, q = s // 8, s % 8
                    nc.tensor.matmul(
                        pd[g * 64:(g + 1) * 64],
                        ones_sb[:, q * 64:(q + 1) * 64],
                        mx[:, s],
                        start=(q == 0),
                        stop=False,
                    )
                # pd += -s_i/2  (identity stationary, j-broadcast moving)
                nc.tensor.matmul(
                    pd[:],
                    ident_sb[:],
                    s_half[:, t, None, :].broadcast_to([128, 32, FO]),
                    start=False,
                    stop=False,
                    skip_group_check=True,
                )
                # pd += -s_j/2  (f8-selector stationary over own j columns)
                nc.tensor.matmul(
                    pd[:],
                    ones_sb[:, 528:656],
                    m_full[:, MI:, :],
                    start=False,
                    stop=True,
                    skip_group_check=True,
                )
                et = ep.tile([128, 32, FO], BF16, tag="et")
                nc.scalar.activation(et[:], pd[:], EXP, bias=zero_b[:], scale=-2.0)
                # straight: sum over the slot's 16 i -> rows of j
                sp = aps.tile([F8, 32, FO], FP32, tag="acc")
                nc.tensor.matmul(
                    sp[:], ones_sb[:, 512:520], et[:], start=True, stop=True,
                )
                nc.scalar.copy(str_stage[:, t], sp[:])
                # mirror: sum over each j-half -> rows of the i-slot
                for h in range(2):
                    nc.vector.reduce_sum(
                        mir_stage[:, t, h, :],
                        et[:, h * 16:(h + 1) * 16, :].rearrange(
                            "p j fo -> p fo j"
                        ),
                        axis=mybir.AxisListType.X,
                    )

            nc.sync.dma_start(ostr_d[:], str_stage[:])
            nc.sync.dma_start(omir_d[:], mir_stage[:])

    nc.finalize()
    return nc


def make_in_maps(x: np.ndarray, T_w: np.ndarray):
    xb = x.astype(NPBF16)
    T_b = np.ascontiguousarray(T_w).astype(NPBF16)
    T_perm = np.ascontiguousarray(
        T_b.reshape(KC, 128, FO, 128).transpose(1, 2, 0, 3)
    ).reshape(128, FO * KC * 128)

    p = np.arange(128)[:, None]
    r = np.arange(F8)[None, :]
    ones_a = (p % 8 == r).astype(NPBF16)                   # [128, 8]
    q = np.arange(64)[None, None, :]
    sq = np.arange(8)[None, :, None]
    ones_k = (q == sq * 8 + p[:, :, None] // 16).astype(NPBF16).reshape(128, 512)
    pad8 = np.zeros((128, 8), NPBF16)
    op = np.arange(128)[None, :]
    sj_stat = (-0.5 * (p // 16 == op % 8)).astype(NPBF16)  # [128, 128]
    ones_pack = np.ascontiguousarray(
        np.concatenate([ones_k, ones_a, pad8, sj_stat], axis=1)
    )

    in_maps = []
    for c in range(N_CORES):
        cols = []
        for a in SLOT_TAB[c]:
            cols.extend(range(a * 16, (a + 1) * 16))
        for b in (c, c + 8):
            cols.extend(range(b * 16, (b + 1) * 16))
        # xT[p, cc, col] = x[cols[col], cc*128+p]
        xc = np.ascontiguousarray(
            xb[cols, :].T.reshape(KC, 128, MCOL).transpose(1, 0, 2)
        )
        in_maps.append({
            "xT": xc,
            "T_w": T_perm,
            "ones_pack": ones_pack,
            "ident": np.eye(128, dtype=np.float32),
        })
    return in_maps


def assemble(x: np.ndarray, strs, mirs) -> np.ndarray:
    """strs: per-core [8, T, 32, FO]; mirs: per-core [128, T, 2, FO]."""
    pair = np.zeros((B, OUT_F), np.float64)
    gq = np.arange(128)
    i_local = (gq // 64) * 8 + (gq // 8) % 8    # partition -> i within slot
    f8_of_p = gq % 8
    for (a, b), (c, t, h, use_mirror, iblk, jblk) in OWN.items():
        st = strs[c]                             # [8, T, 32, FO]
        # straight: rows of jblk get sum over i in iblk
        blk = st[:, t, h * 16:(h + 1) * 16, :]   # [f8, j16, fo]
        pair[jblk * 16:(jblk + 1) * 16, :] += (
            blk.transpose(1, 2, 0).reshape(16, OUT_F)
        )
        if use_mirror:
            mi = mirs[c][:, t, h, :]             # [128(p), fo]
            acc = np.zeros((16, FO, F8))
            acc[i_local, :, f8_of_p] = mi
            pair[iblk * 16:(iblk + 1) * 16, :] += acc.reshape(16, OUT_F)
    pair -= 1.0
    out = np.empty((B, IN_F + OUT_F), np.float32)
    out[:, :IN_F] = x
    out[:, IN_F:] = pair.astype(np.float32)
    return out


_NC_CACHE = None


def kernel(x: np.ndarray, T: np.ndarray) -> np.ndarray:
    global _NC_CACHE
    from concourse import bass_utils

    if _NC_CACHE is None:
        _NC_CACHE = build_nc()
    nc = _NC_CACHE
    in_maps = make_in_maps(np.asarray(x, np.float32), np.asarray(T, np.float32))
    res = bass_utils.run_bass_kernel_spmd(nc, in_maps, core_ids=list(range(N_CORES)))
    strs = [r["out_str"].astype(np.float32) for r in res.results]
    mirs = [r["out_mir"].astype(np.float32) for r in res.results]
    return assemble(np.asarray(x, np.float32), strs, mirs)
